# revision 47
# baseline (speedup 1.0000x reference)
"""Trainium2 Bass kernel for DecouplePreAggGraphConv (GNN message passing).

out[b,j,:] = diag(adj)[j] * (x[b,j] @ W0[j])
           + sum_k offdiag(adj)[j,k] * (x[b,k] @ W1[k])
           + bias

Data-parallel over B across 8 NeuronCores. Host prep pre-transposes x to
[n, (k, b)] bf16 (so the kernel never transposes x on-chip) and
un-permutes the device output from the mix layout (host pre/post is
outside device time, like the weight folding).

phases=0 "folded": out[b,(j,m)] = x[b,(k,n)] @ Mbig + bias as one GEMM
  with contraction (k,n)=2176 (J x the minimal flops, PE-bound).

phases=4 "decoupled" (default): per 128-row tile, software-pipelined
  A. per (k,h): matmul(stationary=W_h[k] [n,128m], moving=xT_k [n,128b])
     -> hT_{h,k} [m, b] PSUM; k-pair-batched drains to SBUF bf16.
  B. per b-triple g: PE transpose with stationary =
     hT[:, 3g:3g+3, :, :] ([m, 102] contiguous cols = (i,h,k)) -> PSUM
     [102, m]; quad-batched drains into M[(i,h,k), (g,m)] bf16.
     M rows 102-104 hold the bias (written once per buffer).
  C. mix GEMM interleaved per quad: stationary mix3 [105, 51]
     (off.T / I / ones blocks), moving M columns -> O[(i,j), (g,m)];
     contiguous store of O; host maps [(i,j),(g,m)] -> [3g+i, j, m].
  Emission is skewed (A of tile t, then B/C of tile t-1) so the PE never
  waits on the drain engines at phase handoffs.
"""

import os
import sys

for _p in ("/opt/trn_rl_repo", "/root/.axon_site/_ro/trn_rl_repo"):
    if os.path.isdir(_p) and _p not in sys.path:
        sys.path.insert(0, _p)

import numpy as np

import concourse.bass as bass
import concourse.mybir as mybir
import concourse.tile as tile
from concourse import bacc
from concourse.bass_utils import run_bass_kernel_spmd

B, J, FIN, FOUT = 16384, 17, 128, 128
N_CORES = 8
TB = 128            # batch rows per tile
CJ = J * FOUT       # 2176
CJ2 = 2304          # CJ padded so chunks are 512-wide PSUM-bank aligned
G3 = TB // 3        # 42 full groups of 3 rows; rows 126/127 ride as group 42
NG = G3 + 1         # 43 group slots (last one only has i=0,1 valid)
HPF = NG * FOUT     # 5504 free size of the mixing moving tile
F32 = mybir.dt.float32
BF16 = mybir.dt.bfloat16

_prog_cache: dict[tuple, object] = {}


def _build_folded(nc, xst, mbig, biasbc, out, bs, repeat):
    """out[b,(j,m)] = x[b,(k,n)] @ Mbig + bias; xT comes pre-transposed."""
    nt = bs // TB
    chunks = [(0, 512), (512, 512), (1024, 512), (1536, 512), (2048, 128)]
    with tile.TileContext(nc) as tc:
        with (
            tc.tile_pool(name="const", bufs=1) as cpool,
            tc.tile_pool(name="x", bufs=3) as xpool,
            tc.tile_pool(name="osb", bufs=2) as opool,
            tc.tile_pool(name="of", bufs=2, space=bass.MemorySpace.PSUM) as ofp,
        ):
            mb_sb = cpool.tile([FIN, J, CJ2], BF16, tag="mbig")
            nc.sync.dma_start(mb_sb[:], mbig[:])
            bb_sb = cpool.tile([TB, CJ], F32, tag="biasbc")
            nc.sync.dma_start(bb_sb[:], biasbc[:])

            for t in range(nt * repeat):
                t = t % nt
                x_t = xpool.tile([FIN, J, TB], BF16, tag="x")
                nc.sync.dma_start(x_t[:], xst[t])

                o_sb = opool.tile([TB, CJ], F32, tag="osb")
                for c0, cw in chunks:
                    of = ofp.tile([TB, 512], F32, tag="of")
                    for k in range(J):
                        nc.tensor.matmul(of[:, :cw], x_t[:, k, :],
                                         mb_sb[:, k, c0:c0 + cw],
                                         start=(k == 0), stop=(k == J - 1))
                    dw = min(cw, CJ - c0)
                    nc.vector.tensor_add(o_sb[:, c0:c0 + dw],
                                         of[:, :dw], bb_sb[:, c0:c0 + dw])
                b0 = t * TB
                nc.sync.dma_start(
                    out[b0:b0 + TB].rearrange("b j m -> b (j m)"), o_sb[:])

    nc.compile()
    return nc


def _build_decoupled(nc, xst, wcat, mix3, bias43, ident, out, bs, repeat,
                     ablate=0):
    """ablate (timing experiments only): 9 = DMA skeleton."""
    nt = bs // TB
    with tile.TileContext(nc) as tc:
        with (
            tc.tile_pool(name="const", bufs=1) as cpool,
            tc.tile_pool(name="x", bufs=3) as xpool,
            tc.tile_pool(name="ht", bufs=2) as htpool,
            tc.tile_pool(name="mm", bufs=2) as mpool,
            tc.tile_pool(name="osb", bufs=2) as opool,
            tc.tile_pool(name="hp", bufs=2, space=bass.MemorySpace.PSUM) as hpp,
            tc.tile_pool(name="tp", bufs=2, space=bass.MemorySpace.PSUM) as tpp,
            tc.tile_pool(name="mx", bufs=2, space=bass.MemorySpace.PSUM) as mxp,
        ):
            # constants
            w_sb = cpool.tile([FIN, J, 2, FOUT], BF16, tag="wcat")
            nc.sync.dma_start(w_sb[:], wcat[:])
            mx_sb = cpool.tile([105, 51], BF16, tag="mix3")
            nc.sync.dma_start(mx_sb[:], mix3[:])
            id_sb = cpool.tile([128, 128], BF16, tag="ident")
            nc.sync.dma_start(id_sb[:], ident[:])

            # M moving tiles: bias rows 102-104 written once per buffer.
            m_bufs = []
            for p in range(2):
                mt = mpool.tile([105, HPF], BF16, tag="M")
                nc.sync.dma_start(mt[102:105, :], bias43[:])
                nc.vector.memset(mt[64:102, G3 * FOUT:], 0.0)
                m_bufs.append(mt)

            def emit_A(step):
                t = step % nt
                x_t = xpool.tile([FIN, J, TB], BF16, tag="x")
                nc.sync.dma_start(x_t[:], xst[t])
                # A. hT[m, b, (h,k)] = W_h[k].T @ xT_k; two k's (4
                # matmuls) share one PSUM bank, one batched drain per pair
                ht_sb = htpool.tile([FOUT, TB, 2, J], BF16, tag="ht")
                for k0 in range(0, J, 2):
                    kw = min(2, J - k0)
                    hp = hpp.tile([FOUT, 2, 2, TB], F32, tag="hp")
                    for dk in range(kw):
                        for h in range(2):
                            nc.tensor.matmul(hp[:, dk, h, :],
                                             w_sb[:, k0 + dk, h, :],
                                             x_t[:, k0 + dk, :])
                    dst = ht_sb[:, :, :, k0:k0 + kw].rearrange(
                        "m b h k -> m k h b")
                    if (k0 // 2) % 2 == 0:
                        nc.vector.tensor_copy(dst, hp[:, :kw])
                    else:
                        nc.scalar.copy(dst, hp[:, :kw])
                return ht_sb

            def emit_BC(step, ht_sb):
                t = step % nt
                mt = m_bufs[step % 2]
                o_sb = opool.tile([51, HPF], BF16, tag="osb")
                ht_flat = ht_sb.rearrange("m b h k -> m (b h k)")
                # B. per 4-group quad: transposes -> one batched drain.
                # Full triples load a 128-col stationary (102 real cols +
                # 26 cols of the next triple, discarded at drain) so FWL
                # engages; only the g=42 tail stays narrow.
                # C helper: mix matmul for one 512-col quad block, two
                # quads per [51,1024] PSUM tile, drained per pair.
                NQ = (NG + 3) // 4
                mps = {}

                def do_mix(q):
                    s0 = q * 512
                    cw = min(512, HPF - s0)
                    if q % 2 == 0:
                        mp_new = mxp.tile([51, 1024], F32, tag="mx")
                        mps[q // 2] = mp_new
                    mp = mps[q // 2]
                    nc.tensor.matmul(mp[:, (q % 2) * 512:(q % 2) * 512 + cw],
                                     mx_sb[:], mt[:, s0:s0 + cw])
                    if q % 2 == 1 or q == NQ - 1:
                        d0 = (q // 2) * 1024
                        dw = (q % 2) * 512 + cw
                        if (q // 2) % 2 == 0:
                            nc.vector.tensor_copy(o_sb[:, d0:d0 + dw],
                                                  mp[:, :dw])
                        else:
                            nc.scalar.copy(o_sb[:, d0:d0 + dw], mp[:, :dw])

                # B. transposes per quad; mix matmuls ride 2 quads behind
                # (their input drains are long complete, and the real
                # matmuls keep the PE HAM-warm through the transpose run).
                for g0 in range(0, NG, 4):
                    gw = min(4, NG - g0)
                    q = g0 // 4
                    tp = tpp.tile([128, 4, FOUT], BF16, tag="tp")
                    for dg in range(gw):
                        g = g0 + dg
                        if g < NG - 1:
                            nc.tensor.transpose(
                                tp[:, dg, :],
                                ht_flat[:, 102 * g:102 * g + 128], id_sb[:])
                        else:
                            nc.tensor.transpose(
                                tp[:68, dg, :],
                                ht_flat[:, 102 * g:102 * g + 68], id_sb[:])
                    # g=42 fills only 68 rows; rows 68-101 drain stale
                    # PSUM into M slots feeding never-stored outputs.
                    dst = mt[0:102, g0 * FOUT:(g0 + gw) * FOUT].rearrange(
                        "p (g m) -> p g m", g=gw)
                    if q % 2 == 0:
                        nc.vector.tensor_copy(dst, tp[:102, :gw, :])
                    else:
                        nc.scalar.copy(dst, tp[:102, :gw, :])
                    if q >= 2:
                        do_mix(q - 2)
                do_mix(NQ - 2)
                do_mix(NQ - 1)
                # contiguous bf16 store in mix layout; host un-permutes
                nc.sync.dma_start(out[t], o_sb[:])

            if ablate == 9:
                for step in range(nt * repeat):
                    t = step % nt
                    x_t = xpool.tile([FIN, J, TB], BF16, tag="x")
                    nc.sync.dma_start(x_t[:], xst[t])
                    o_sb = opool.tile([51, HPF], F32, tag="osb")
                    nc.vector.memset(o_sb[:, 0:2], 0.0)
                    nc.sync.dma_start(out[t], o_sb[:])
            else:
                for step in range(nt * repeat):
                    ht_sb = emit_A(step)
                    emit_BC(step, ht_sb)

    nc.compile()
    return nc


def _build_program(bs: int, repeat: int = 1, phases: int = 0):
    nt = bs // TB
    assert bs % TB == 0

    nc = bacc.Bacc("TRN2", target_bir_lowering=False, debug=False,
                   num_devices=N_CORES)

    xst = nc.declare_dram_parameter("xst", [nt, FIN, J * TB], BF16,
                                    isOutput=False)
    xst = xst.rearrange("t n (k b) -> t n k b", k=J)

    if phases == 0:
        out = nc.declare_dram_parameter("out", [bs, J, FOUT], F32,
                                        isOutput=True)
        mbig = nc.declare_dram_parameter("mbig", [FIN, J, CJ2], BF16,
                                         isOutput=False)
        biasbc = nc.declare_dram_parameter("biasbc", [TB, CJ], F32,
                                           isOutput=False)
        return _build_folded(nc, xst, mbig, biasbc, out, bs, repeat)

    # decoupled: device emits the mix layout [t, (i j), (g m)] in bf16;
    # host upconverts and un-permutes to [b, j, m] f32.
    out = nc.declare_dram_parameter("out", [nt, 51, HPF], BF16,
                                    isOutput=True)
    wcat = nc.declare_dram_parameter("wcat", [FIN, J, 2 * FOUT], BF16,
                                     isOutput=False)
    wcat = wcat.rearrange("n k (h m) -> n k h m", h=2)
    mix3 = nc.declare_dram_parameter("mix3", [105, 51], BF16, isOutput=False)
    bias43 = nc.declare_dram_parameter("bias43", [3, HPF], BF16,
                                       isOutput=False)
    ident = nc.declare_dram_parameter("ident", [128, 128], BF16,
                                      isOutput=False)
    return _build_decoupled(nc, xst, wcat, mix3, bias43, ident, out, bs,
                            repeat, ablate=(9 if phases == 9 else 0))


def _host_prep(x, W, bias, adj, bs):
    """Build the per-core input maps (pure numpy; outside device time)."""
    import ml_dtypes
    BF = ml_dtypes.bfloat16
    diag = np.diagonal(adj).astype(np.float32)
    off = (adj * (1.0 - np.eye(J, dtype=adj.dtype))).astype(np.float32)

    # stage-A weights, n-partition-major: [FIN, J, 2, FOUT],
    # [:, k, 0] = diag_k*W0_k, [:, k, 1] = W1_k
    wcat = np.stack([diag[:, None, None] * W[0], W[1]], axis=1)  # [J,2,n,m]
    wcat = np.ascontiguousarray(wcat.transpose(2, 0, 1, 3)).reshape(
        FIN, J, 2 * FOUT).astype(BF)

    # mixing stationary: rows r = i*34 + h*17 + k (h=0: h0s, h=1: h1),
    # rows 102+i: bias; cols (i*17 + j)
    mixblock = np.zeros((34, J), dtype=np.float32)
    mixblock[0:J, :] = np.eye(J, dtype=np.float32)
    mixblock[J:2 * J, :] = off.T
    mix3 = np.zeros((105, 51), dtype=np.float32)
    for i in range(3):
        mix3[i * 34:(i + 1) * 34, i * J:(i + 1) * J] = mixblock
        mix3[102 + i, i * J:(i + 1) * J] = 1.0
    bias43 = np.tile(bias.astype(np.float32), (3, NG))

    # folded weights: Mbig[(k,n),(j,m)], stored n-partition-major
    m4 = off.T[:, :, None, None] * W[1][:, None, :, :]   # [k, j, n, m]
    m4[np.arange(J), np.arange(J)] += diag[:, None, None] * W[0]
    mbig = m4.transpose(0, 2, 1, 3).reshape(J * FIN, CJ)  # rows (k,n)
    mbig = np.ascontiguousarray(
        mbig.reshape(J, FIN, CJ).transpose(1, 0, 2)).astype(np.float32)
    mbig = np.concatenate(
        [mbig, np.zeros((FIN, J, CJ2 - CJ), np.float32)], axis=2)

    shared = {
        "wcat": wcat,
        "mix3": mix3.astype(BF),
        "bias43": np.ascontiguousarray(bias43).astype(BF),
        "ident": np.eye(128, dtype=np.float32).astype(BF),
        "mbig": mbig.astype(BF),
        "biasbc": np.ascontiguousarray(np.broadcast_to(
            np.tile(bias.astype(np.float32), 17), (TB, CJ))),
    }
    # x pre-transposed + pre-tiled: [nt, FIN, J*TB]; per-partition rows
    # are contiguous. xt[t, n, k*TB + bb] = x[t*TB + bb, k, n]
    nt = bs // TB
    in_maps = []
    for c in range(N_CORES):
        xs = x[c * bs:(c + 1) * bs].astype(BF)          # [bs, J, FIN]
        xt = xs.reshape(nt, TB, J, FIN).transpose(0, 3, 2, 1)
        m = dict(shared)
        m["xst"] = np.ascontiguousarray(xt).reshape(nt, FIN, J * TB)
        in_maps.append(m)
    return in_maps


def _run(x, W, bias, adj, bs, profile=False, tmpdir=None, phases=4):
    key = (bs, phases)
    if key not in _prog_cache:
        _prog_cache[key] = _build_program(bs, phases=phases)
    nc = _prog_cache[key]
    in_maps = _host_prep(x, W, bias, adj, bs)
    res = run_bass_kernel_spmd(nc, in_maps, list(range(N_CORES)),
                               trace=profile, tmpdir=tmpdir)
    nt = bs // TB
    outs = []
    for c in range(N_CORES):
        o = res.results[c]["out"]
        if phases != 0:
            # [t, (i j), (g m)] bf16 -> [t*128 + 3g + i, j, m] f32
            o = np.asarray(o).astype(np.float32)
            o = o.reshape(nt, 3, J, NG, FOUT).transpose(0, 3, 1, 2, 4)
            o = np.ascontiguousarray(o).reshape(nt, 3 * NG, J, FOUT)
            o = o[:, :TB].reshape(bs, J, FOUT)
        outs.append(o)
    out = np.concatenate(outs, axis=0)
    if profile:
        return out, res
    return out


def kernel(x, W, bias, adj):
    x = np.asarray(x, dtype=np.float32)
    W = np.asarray(W, dtype=np.float32)
    bias = np.asarray(bias, dtype=np.float32)
    adj = np.asarray(adj, dtype=np.float32)
    assert x.shape == (B, J, FIN)
    return _run(x, W, bias, adj, B // N_CORES, phases=PHASES)


PHASES = 4
